# revision 42
# baseline (speedup 1.0000x reference)
"""Trainium2 Bass kernel for nn_LorentzGNN (2x GATv2 + Lorentz head), 8-core SPMD.

Sharding: nodes (and their in-edges) are partitioned contiguously across 8 cores
(2048 nodes each). The layer-1 source transform (xl1 table) is REPLICATED on
every core (cheaper than an AllGather of the table); layer-2 takes one
AllGather of xl2. Per-edge work uses gpsimd dma_gather (edge-major + feat-major
transposed) and host-built 0/1 segment matrices (fp8) driven through the PE as
matmuls. Leaky-relu is a single Prelu activation (co-resident with Exp in one
act table -> no table reloads in the edge phase). Gelu and the Lorentz sqrt are
deferred to batched passes for the same reason. Graph phase (centroid + Lorentz
MLP over the 16 graphs each core owns) is local; host concatenates.
"""
import numpy as np
import ml_dtypes

# ---------- problem constants (hardcoded per contract) ----------
N, E, B = 16384, 131072, 128
FT, HEADS, C = 512, 4, 128
NCORES = 8
SHARD = N // NCORES            # 2048
P = 128
NT = SHARD // P                # 16 dst tiles per core
NTALL = N // P                 # 128 tiles overall
GPC = B // NCORES              # 16 graphs per core
LEAK = 0.2

_cache = {}


# ============================ host-side prep ============================

def _prep_edges(edge_index):
    src = np.concatenate([edge_index[0], np.arange(N)]).astype(np.int64)
    dst = np.concatenate([edge_index[1], np.arange(N)]).astype(np.int64)
    order = np.argsort(dst, kind="stable")
    src, dst = src[order], dst[order]
    counts = np.bincount(dst // P, minlength=NTALL)
    LP = int(np.ceil(max(counts.max(), 1) / 128) * 128)
    NJ = LP // P
    srcs = np.zeros((NTALL, LP), np.int16)
    dloc = np.full((NTALL, LP), -1, np.int32)
    starts = np.concatenate([[0], np.cumsum(counts)])
    for t in range(NTALL):
        c = counts[t]
        srcs[t, :c] = src[starts[t]:starts[t] + c]
        dloc[t, :c] = dst[starts[t]:starts[t] + c] - t * P
    # segment matrices
    sdt = np.zeros((NTALL, P, LP), np.float16)           # [dst, j]
    jj = np.arange(LP)
    for t in range(NTALL):
        v = dloc[t] >= 0
        sdt[t, dloc[t, v], jj[v]] = 1.0
    sjt = np.ascontiguousarray(sdt.transpose(0, 2, 1))   # [j, dst] edge-major
    # idx buffers wrapped in 16 partitions, replicated to 128
    idx = srcs.reshape(NTALL, LP // 16, 16).transpose(0, 2, 1)  # [t, 16, LP/16]
    idx = np.tile(idx, (1, 8, 1)).astype(np.int16)              # [t, 128, LP/16]
    return srcs, sdt, sjt, idx, LP, NJ


def _interleave_k(w, kchunks):
    """[K*128, N] -> [128, K, N] host layout for SBUF."""
    K, Nn = w.shape
    assert K == kchunks * 128
    return np.ascontiguousarray(w.reshape(kchunks, 128, Nn).transpose(1, 0, 2))


def _aug_w(W, b, kpad, npad=None):
    """stack rows [W; b; 0-pad] to kpad rows, optionally pad cols to npad."""
    K, Nn = W.shape
    out = np.zeros((kpad, Nn if npad is None else npad), np.float32)
    out[:K, :Nn] = W
    out[K, :Nn] = b
    return out


# ============================ kernel build ============================

def _build(LP, NJ, ex_dtype_name):
    import contextlib
    import concourse.bass as bass  # noqa
    import concourse.bacc as bacc
    import concourse.tile as tile
    from concourse import mybir
    from concourse.library_config import mlp as gpsimd_mlp

    f32, f16 = mybir.dt.float32, mybir.dt.float16
    bf16, i16 = mybir.dt.bfloat16, mybir.dt.int16
    f8 = mybir.dt.float8e4
    EXD = {"float16": f16, "bfloat16": bf16}[ex_dtype_name]
    AF = mybir.ActivationFunctionType
    ALU = mybir.AluOpType
    SC = [(o, min(512, LP - o)) for o in range(0, LP, 512)]

    nc = bacc.Bacc("TRN2", target_bir_lowering=False, debug=False,
                   num_devices=NCORES)
    groups = [list(range(NCORES))]

    # ---- DRAM I/O (per-core, same program) ----
    xfT = nc.dram_tensor("xfT", [128, 5 * N], f16, kind="ExternalInput")
    xoT = nc.dram_tensor("xoT", [128, 5 * SHARD], f16, kind="ExternalInput")
    w1l = nc.dram_tensor("w1l", [128, 5 * FT], f16, kind="ExternalInput")
    w1r = nc.dram_tensor("w1r", [128, 5 * FT], f16, kind="ExternalInput")
    w2l = nc.dram_tensor("w2l", [128, 4 * FT], f16, kind="ExternalInput")
    w2r = nc.dram_tensor("w2r", [128, 4 * FT], f16, kind="ExternalInput")
    b2r_d = nc.dram_tensor("b2rows", [2, FT], f16, kind="ExternalInput")
    a1_d = nc.dram_tensor("a1", [128, 4 * HEADS], f16, kind="ExternalInput")
    a2_d = nc.dram_tensor("a2", [128, 4 * HEADS], f16, kind="ExternalInput")
    b1f_d = nc.dram_tensor("b1full", [128, FT], f16, kind="ExternalInput")
    b2f_d = nc.dram_tensor("b2full", [128, FT], f16, kind="ExternalInput")
    sdt_d = nc.dram_tensor("sdt", [NT, 128, LP], f8, kind="ExternalInput")
    sj_d = nc.dram_tensor("sj", [NT, 128, NJ * 128], f8, kind="ExternalInput")
    idx_d = nc.dram_tensor("idx", [NT, 128, LP // 16], i16, kind="ExternalInput")
    ecols_d = nc.dram_tensor("ecols", [128, NT * GPC], f16, kind="ExternalInput")
    ident_d = nc.dram_tensor("ident", [128, 128], f16, kind="ExternalInput")
    wa_d = nc.dram_tensor("wa", [128, 5 * 2560], f16, kind="ExternalInput")
    wb_d = nc.dram_tensor("wb", [128, 17 * 640], f16, kind="ExternalInput")
    wf_d = nc.dram_tensor("wf", [128, 5 * 640], f16, kind="ExternalInput")
    sabf_d = nc.dram_tensor("sabf", [16, 3], f32, kind="ExternalInput")

    xl1_tb = nc.dram_tensor("xl1_tb", [N, FT], f16)
    xl2_sh = nc.dram_tensor("xl2_sh", [SHARD, FT], f16)
    xl2_tb = nc.dram_tensor("xl2_tb", [N, FT], f16, addr_space="Shared")
    z0_d = nc.dram_tensor("z0buf", [GPC, FT + 1], f16)
    za_d = nc.dram_tensor("zabuf", [1, GPC], f32)
    zout = nc.dram_tensor("zout", [GPC, FT + 1], f32, kind="ExternalOutput")
    gmout = nc.dram_tensor("gmout", [GPC, FT + 1], f32, kind="ExternalOutput")

    with tile.TileContext(nc, num_cores=NCORES) as tc:
        est = contextlib.ExitStack()
        with est:
            nc.gpsimd.load_library(gpsimd_mlp)
            nregs = {n: nc.gpsimd.to_reg(n) for n in sorted({n for _, n in SC})}
            cpool = est.enter_context(tc.tile_pool(name="consts", bufs=1))
            resp = est.enter_context(tc.tile_pool(name="res", bufs=1))
            mearly = est.enter_context(tc.tile_pool(name="mearly", bufs=1))
            big = contextlib.ExitStack()
            xsp = big.enter_context(tc.tile_pool(name="xstream", bufs=2))
            sbp = big.enter_context(tc.tile_pool(name="stream", bufs=2))
            smp = big.enter_context(tc.tile_pool(name="small", bufs=2))
            psb = big.enter_context(tc.tile_pool(name="psb", bufs=2, space="PSUM"))
            pgp = big.enter_context(tc.tile_pool(name="pgp", bufs=1, space="PSUM"))
            pss = big.enter_context(tc.tile_pool(name="pss", bufs=2, space="PSUM"))

            # ---- consts ----
            def cload(name, shape, dt_, src):
                t = cpool.tile(shape, dt_, name=name)
                nc.sync.dma_start(t[:], src)
                return t

            w1l_s = cload("w1l_s", [128, 5 * FT], f16, w1l[:])
            w1r_s = cload("w1r_s", [128, 5 * FT], f16, w1r[:])
            w2l_s = cload("w2l_s", [128, 4 * FT], f16, w2l[:])
            w2r_s = cload("w2r_s", [128, 4 * FT], f16, w2r[:])
            b2la_s = cload("b2la_s", [1, FT], f16, b2r_d[0:1, :])
            b2ra_s = cload("b2ra_s", [1, FT], f16, b2r_d[1:2, :])
            a1_s = cload("a1_s", [128, 4 * HEADS], f16, a1_d[:])
            a2_s = cload("a2_s", [128, 4 * HEADS], f16, a2_d[:])
            b1f_s = cload("b1f_s", [128, FT], f16, b1f_d[:])
            b2f_s = cload("b2f_s", [128, FT], f16, b2f_d[:])
            ident_s = cload("ident_s", [128, 128], f16, ident_d[:])
            ecols_s = cload("ecols_s", [128, NT * GPC], f16, ecols_d[:])
            ones1 = cpool.tile([1, FT], f16, name="ones1")
            nc.vector.memset(ones1[:], 1.0)
            onesc = cpool.tile([128, 1], f32, name="onesc")
            nc.vector.memset(onesc[:], 1.0)

            def nsqrt(out_ap, x_ap, pool, pfx):
                """out = sqrt(x), Newton-refined (ACT sqrt LUT is ~4e-3)."""
                y0 = pool.tile(list(x_ap.shape), f32, tag="nsq", name=pfx + "y0",
                               bufs=6)
                nc.scalar.activation(y0[:], x_ap, AF.Sqrt)
                r0 = pool.tile(list(x_ap.shape), f32, tag="nsq", name=pfx + "r0",
                               bufs=6)
                nc.vector.reciprocal(r0[:], y0[:])
                nc.vector.tensor_tensor(out=r0[:], in0=x_ap, in1=r0[:],
                                        op=ALU.mult)
                nc.vector.tensor_tensor(out=y0[:], in0=y0[:], in1=r0[:],
                                        op=ALU.add)
                nc.vector.tensor_scalar_mul(out_ap, y0[:], 0.5)

            # resident tiles
            xr_s = resp.tile([128, NT * FT], f16, name="xr_s")
            h1T_s = resp.tile([128, NT * FT], f16, name="h1T_s")
            h2_s = resp.tile([128, NT * (FT + 1)], f16, name="h2_s")
            acb_s = resp.tile([128, NT], f32, name="acb_s")
            h2_v = h2_s[:].rearrange("p (t c) -> p t c", t=NT)

            # ---------------- transform1: replicated xl1 table ----------------
            def transform1():
                w1l_v = w1l_s[:].rearrange("p (k n) -> p k n", k=5)
                w1r_v = w1r_s[:].rearrange("p (k n) -> p k n", k=5)
                xfT_v = xfT[:].rearrange("p (k n) -> p k n", k=5)
                CH = 4  # tiles per streamed chunk
                for c0 in range(0, NTALL, CH):
                    xc = xsp.tile([128, 5 * CH * 128], f16, tag="xc", name="xc")
                    xc_v = xc[:].rearrange("p (k n) -> p k n", k=5)
                    nc.sync.dma_start(xc_v, xfT_v[:, :, c0 * 128:(c0 + CH) * 128])
                    for ti in range(CH):
                        t = c0 + ti
                        pz2 = psb.tile([128, 1024], f32, tag="pz2", name="pl1")
                        pl = pz2[:, 0:FT]
                        for kc in range(5):
                            lh = xc_v[:, kc, ti * 128:(ti + 1) * 128]
                            nc.tensor.matmul(pl, lhsT=lh, rhs=w1l_v[:, kc, :],
                                             start=(kc == 0), stop=(kc == 4))
                        xlt = smp.tile([128, FT], f16, tag="xlt", name="xlt",
                                       bufs=8)
                        if t % 2 == 0:
                            nc.scalar.activation(xlt[:], pl, AF.Copy)
                        else:
                            nc.vector.tensor_copy(xlt[:], pl)
                        nc.sync.dma_start(xl1_tb[t * 128:(t + 1) * 128, :], xlt[:])
                # own-shard xr1 from the per-core xoT input
                xoT_v = xoT[:].rearrange("p (k n) -> p k n", k=5)
                for c0 in range(0, NT, CH):
                    xc = xsp.tile([128, 5 * CH * 128], f16, tag="xc", name="xco")
                    xc_v = xc[:].rearrange("p (k n) -> p k n", k=5)
                    nc.sync.dma_start(xc_v, xoT_v[:, :, c0 * 128:(c0 + CH) * 128])
                    for ti in range(CH):
                        t = c0 + ti
                        pz2 = psb.tile([128, 1024], f32, tag="pz2", name="pr1")
                        pr = pz2[:, 0:FT]
                        for kc in range(5):
                            lh = xc_v[:, kc, ti * 128:(ti + 1) * 128]
                            nc.tensor.matmul(pr, lhsT=lh, rhs=w1r_v[:, kc, :],
                                             start=(kc == 0), stop=(kc == 4))
                        if t % 2 == 0:
                            nc.scalar.activation(xr_s[:, t * FT:(t + 1) * FT],
                                                 pr, AF.Copy)
                        else:
                            nc.vector.tensor_copy(xr_s[:, t * FT:(t + 1) * FT],
                                                  pr)

            # ---------------- transform2 (local shard) ----------------
            # h1T_s is produced (pre-gelu) inline in edge_layer(1); a batched
            # gelu pass then runs, interleaved with transform2_xl.
            def transform2_xl(tiles):
                """xl2 = h1T.T @ W2l + bl2 -> xl2_sh."""
                w2l_v = w2l_s[:].rearrange("p (k n) -> p k n", k=4)
                for t in tiles:
                    h1T_v = h1T_s[:, t * FT:(t + 1) * FT].rearrange(
                        "p (k n) -> p k n", k=4)
                    pz2 = psb.tile([128, 1024], f32, tag="pz2", name="pl2")
                    pl = pz2[:, 0:FT]
                    for kc in range(4):
                        nc.tensor.matmul(pl, lhsT=h1T_v[:, kc, :],
                                         rhs=w2l_v[:, kc, :],
                                         start=(kc == 0), stop=False)
                    nc.tensor.matmul(pl, lhsT=ones1[:, 0:128], rhs=b2la_s[:],
                                     start=False, stop=True)
                    xlt = smp.tile([128, FT], f16, tag="xlt", name="xlt2",
                                   bufs=8)
                    if t % 2 == 0:
                        nc.scalar.activation(xlt[:], pl, AF.Copy)
                    else:
                        nc.vector.tensor_copy(xlt[:], pl)
                    nc.sync.dma_start(xl2_sh[t * 128:(t + 1) * 128, :], xlt[:])

            def transform2_xr():
                """xr2 = h1T.T @ W2r + br2 -> xr_s (overlaps the AllGather)."""
                w2r_v = w2r_s[:].rearrange("p (k n) -> p k n", k=4)
                for t in range(NT):
                    h1T_v = h1T_s[:, t * FT:(t + 1) * FT].rearrange(
                        "p (k n) -> p k n", k=4)
                    pz2 = psb.tile([128, 1024], f32, tag="pz2", name="pr2")
                    pr = pz2[:, 0:FT]
                    for kc in range(4):
                        nc.tensor.matmul(pr, lhsT=h1T_v[:, kc, :],
                                         rhs=w2r_v[:, kc, :],
                                         start=(kc == 0), stop=False)
                    nc.tensor.matmul(pr, lhsT=ones1[:, 0:128], rhs=b2ra_s[:],
                                     start=False, stop=True)
                    if t % 2 == 0:
                        nc.scalar.activation(xr_s[:, t * FT:(t + 1) * FT], pr,
                                             AF.Copy)
                    else:
                        nc.vector.tensor_copy(xr_s[:, t * FT:(t + 1) * FT], pr)

            # ---------------- GATv2 edge layer ----------------
            def edge_layer(layer, table, a_s, bf_s):
                for t in range(NT):
                    idxt = smp.tile([128, LP // 16], i16, tag="idxt", name="idxt")
                    nc.sync.dma_start(idxt[:], idx_d[t, :, :])
                    sdtt = sbp.tile([128, LP], f8, tag="sdtt", name="sdtt")
                    nc.sync.dma_start(sdtt[:], sdt_d[t, :, :])
                    sjt = sbp.tile([128, NJ * 128], f8, tag="sjt", name="sjt")
                    nc.sync.dma_start(sjt[:], sj_d[t, :, :])
                    sj_v = sjt[:].rearrange("p (j d) -> p j d", j=NJ)

                    # gathers: xlgT feat-major (SC-block layout [4c, n]),
                    # xlg edge-major [jb, 512]
                    xlgT = sbp.tile([128, 4 * LP], f16, tag="xlgT", name="xlgT")
                    xlg = sbp.tile([128, NJ * FT], f16, tag="xlg", name="xlg")
                    xlg_w = xlg[:].rearrange("p (j n) -> p j n", j=NJ)
                    for (o, n) in SC:
                        nc.gpsimd.dma_gather(
                            xlgT[:, 4 * o:4 * (o + n)].rearrange(
                                "p (c j) -> p c j", c=4), table[:],
                            idxt[:, o // 16:(o + n) // 16], n, nregs[n], FT,
                            transpose=True)
                        nc.gpsimd.dma_gather(
                            xlg_w[:, o // 128:(o + n) // 128, :], table[:],
                            idxt[:, o // 16:(o + n) // 16], n, nregs[n], FT)
                    xlg_v = xlg[:].rearrange("p (j n) -> p j n", j=NJ)

                    xr_t = xr_s[:].rearrange("p (t n) -> p t n", t=NT)[:, t, :]
                    # z = xl[src] + xr[dst]; fc-chunks paired into one 2-bank
                    # psum so each Prelu covers 2n columns.
                    lr = sbp.tile([128, 4 * LP], f16, tag="lr", name="lr")
                    for (o, n) in SC:
                        for fcp in (0, 2):
                            pz2 = psb.tile([128, 1024], f32, tag="pz2",
                                           name="pz2")
                            for fi in range(2):
                                fc = fcp + fi
                                pz = pz2[:, fi * 512:fi * 512 + n]
                                nc.tensor.matmul(
                                    pz, lhsT=xr_t[:, fc * 128:(fc + 1) * 128],
                                    rhs=sdtt[:, o:o + n], start=True, stop=False)
                                nc.tensor.matmul(
                                    pz, lhsT=ident_s[:],
                                    rhs=xlgT[:, 4 * o + fc * n:
                                             4 * o + (fc + 1) * n],
                                    start=False, stop=True)
                            if n == 512:
                                nc.scalar.activation(
                                    lr[:, 4 * o + fcp * n:4 * o + (fcp + 2) * n],
                                    pz2[:], AF.Prelu, alpha=LEAK)
                            else:
                                for fi in range(2):
                                    fc = fcp + fi
                                    nc.scalar.activation(
                                        lr[:, 4 * o + fc * n:4 * o + (fc + 1) * n],
                                        pz2[:, fi * 512:fi * 512 + n],
                                        AF.Prelu, alpha=LEAK)
                    # attention logits: att is head-block-diagonal, so chunk fc
                    # contributes only to head h=fc -> single-column matmuls.
                    a_v = a_s[:].rearrange("p (c h) -> p c h", c=4)
                    ps_sm = pss.tile([128, 512], f32, tag="ps", name="ps_sm")
                    pe_all = ps_sm[:, 0:NJ * HEADS]
                    pden = ps_sm[:, 128:128 + HEADS]
                    for jb in range(NJ):
                        o = 512 * (jb // 4)
                        n = min(512, LP - o)
                        joff = (jb % 4) * 128
                        for h in range(HEADS):
                            lblk = lr[:, 4 * o + h * n + joff:
                                      4 * o + h * n + joff + 128]
                            nc.tensor.matmul(
                                pe_all[:, jb * HEADS + h:jb * HEADS + h + 1],
                                lhsT=lblk, rhs=a_v[:, h, h:h + 1],
                                start=True, stop=True)
                    exf = smp.tile([128, NJ * HEADS], f32, tag="exf", name="exf")
                    nc.scalar.activation(exf[:], pe_all, AF.Exp)
                    exb = smp.tile([128, NJ * HEADS], EXD, tag="exb", name="exb")
                    nc.vector.tensor_copy(exb[:], exf[:])

                    pagg = pgp.tile([128, FT], f32, tag="pagg", name="pagg",
                                    bufs=1)
                    for jb in range(NJ):
                        wt = sbp.tile([128, FT], EXD, tag="wt", name="wt")
                        for h in range(HEADS):
                            nc.vector.tensor_scalar_mul(
                                wt[:, h * C:(h + 1) * C],
                                xlg_v[:, jb, h * C:(h + 1) * C],
                                exf[:, jb * HEADS + h:jb * HEADS + h + 1])
                        nc.tensor.matmul(pagg[:], lhsT=sj_v[:, jb], rhs=wt[:],
                                         start=(jb == 0), stop=(jb == NJ - 1))
                        nc.tensor.matmul(pden, lhsT=sj_v[:, jb],
                                         rhs=exb[:, jb * HEADS:(jb + 1) * HEADS],
                                         start=(jb == 0), stop=(jb == NJ - 1))
                    rden = smp.tile([128, HEADS], f32, tag="rden", name="rden")
                    nc.vector.reciprocal(rden[:], pden)
                    o1 = smp.tile([128, FT], f16, tag="o1", name="o1")
                    for h in range(HEADS):
                        nc.vector.tensor_scalar_mul(
                            o1[:, h * C:(h + 1) * C], pagg[:, h * C:(h + 1) * C],
                            rden[:, h:h + 1])
                    if layer == 1:
                        # h1pre = o1 + b1; transpose into h1T_s inline (gelu
                        # deferred to a batched pass over h1T_s)
                        h1p = smp.tile([128, FT], f16, tag="h1p", name="h1p")
                        nc.vector.tensor_tensor(out=h1p[:], in0=o1[:],
                                                in1=bf_s[:], op=ALU.add)
                        for fc in range(4):
                            pt = psb.tile([128, 128], f16, tag="pt", name="pt",
                                          bufs=1)
                            nc.tensor.transpose(
                                pt[:], h1p[:, fc * 128:(fc + 1) * 128],
                                ident_s[:])
                            nc.vector.tensor_copy(
                                h1T_s[:, t * FT + fc * 128:
                                      t * FT + (fc + 1) * 128], pt[:])
                    else:
                        nc.vector.tensor_tensor(
                            out=h2_v[:, t, 1:FT + 1], in0=o1[:], in1=bf_s[:],
                            op=ALU.add)
                        sq = smp.tile([128, FT], f16, tag="sq", name="sq")
                        acc = smp.tile([128, 1], f32, tag="acc", name="acc")
                        nc.scalar.activation(sq[:], h2_v[:, t, 1:FT + 1],
                                             AF.Square, accum_out=acc[:])
                        nc.vector.tensor_scalar_add(acb_s[:, t:t + 1], acc[:],
                                                    1.0)
                        # z0 spatial part: row 0 of this tile == graph t's
                        # first node
                        nc.sync.dma_start(z0_d[t:t + 1, 1:FT + 1],
                                          h2_v[0:1, t, 1:FT + 1])

            # ---------------- program ----------------
            transform1()
            edge_layer(1, xl1_tb, a1_s, b1f_s)
            # gelu (on the transposed copy, one table load) interleaved with
            # the xl2 transform so the AllGather can issue as early as possible
            GB = 4 * FT
            for g in range(4):
                nc.scalar.activation(h1T_s[:, g * GB:(g + 1) * GB],
                                     h1T_s[:, g * GB:(g + 1) * GB], AF.Gelu)
                transform2_xl(range(4 * g, 4 * g + 4))
            nc.gpsimd.collective_compute(
                "AllGather", mybir.AluOpType.bypass, replica_groups=groups,
                ins=[xl2_sh[:]], outs=[xl2_tb[:]])
            # prefetch MLP weights + overlap xr2 under the AllGather
            wa_s = mearly.tile([128, 5 * 2560], f16, name="wa_s")
            nc.sync.dma_start(wa_s[:], wa_d[:])
            wf_s = mearly.tile([128, 5 * 640], f16, name="wf_s")
            nc.sync.dma_start(wf_s[:], wf_d[:])
            sabf = mearly.tile([16, 3], f32, name="sabf")
            nc.sync.dma_start(sabf[:], sabf_d[:])
            transform2_xr()
            edge_layer(2, xl2_tb, a2_s, b2f_s)
            # deferred Lorentz time coordinate: t = sqrt(1 + |sp|^2)
            tvals = smp.tile([128, NT], f32, tag="tv", name="tvals")
            nsqrt(tvals[:], acb_s[:], smp, "t_")
            # z0 time col: stage row 0 of tvals through DRAM ([1,16] store,
            # then a standard partition-scattering [16,1] load)
            nc.sync.dma_start(za_d[:], tvals[0:1, :])
            zt16 = smp.tile([GPC, 1], f32, tag="in1", name="zt16")
            nc.sync.dma_start(
                zt16[:], za_d[:].rearrange("o (g x) -> (o g) x", x=1))
            zt16h = smp.tile([GPC, 1], f16, tag="in1h", name="zt16h")
            nc.vector.tensor_copy(zt16h[:], zt16[:])
            nc.sync.dma_start(z0_d[:, 0:1], zt16h[:])

            # ---------------- graph phase: centroid ----------------
            pgm = pgp.tile([128, FT], f32, tag="pagg", name="pgm", bufs=1)
            for t in range(NT):
                ec = ecols_s[:, t * GPC:(t + 1) * GPC]
                nc.tensor.matmul(pgm[:GPC, :], lhsT=ec, rhs=h2_v[:, t, 1:FT + 1],
                                 start=(t == 0), stop=(t == NT - 1))
            # time-coordinate sums: graph g == tile g, so they are the column
            # sums of tvals
            pgm1 = pss.tile([128, 512], f32, tag="ps", name="pgm1")
            nc.tensor.matmul(pgm1[0:NT, 0:1], lhsT=tvals[:], rhs=onesc[:],
                             start=True, stop=True)
            sums = smp.tile([GPC, FT + 1], f32, tag="sums", name="sums")
            nc.scalar.activation(sums[:, 1:FT + 1], pgm[:GPC, :], AF.Copy)
            nc.scalar.activation(sums[:, 0:1], pgm1[:GPC, 0:1], AF.Copy)
            sqg = smp.tile([GPC, FT], f32, tag="sqg", name="sqg")
            sa_ = smp.tile([GPC, 1], f32, tag="acc", name="sa_")
            nc.scalar.activation(sqg[:], sums[:, 1:FT + 1], AF.Square,
                                 accum_out=sa_[:])
            innr = smp.tile([GPC, 1], f32, tag="in1", name="innr")
            nc.vector.tensor_tensor(out=innr[:], in0=sums[:, 0:1],
                                    in1=sums[:, 0:1], op=ALU.mult)
            nc.vector.tensor_tensor(out=innr[:], in0=innr[:], in1=sa_[:],
                                    op=ALU.subtract)
            nc.vector.tensor_scalar_max(innr[:], innr[:], 1e-8 * (N // B) ** 2)
            rt = smp.tile([GPC, 1], f32, tag="in1", name="rt")
            nsqrt(rt[:], innr[:], smp, "g_")
            nc.vector.reciprocal(rt[:], rt[:])
            gmt = smp.tile([GPC, FT + 1], f32, tag="sums", name="gmt")
            nc.scalar.activation(gmt[:], sums[:], AF.Copy, scale=rt[:])
            nc.sync.dma_start(gmout[:], gmt[:])

            # free the big stream/psum pools before the MLP pools allocate
            big.close()

            # ---------------- Lorentz MLP on z0 [16, 513] ----------------
            mpool = est.enter_context(tc.tile_pool(name="mlp", bufs=1))
            msb = est.enter_context(tc.tile_pool(name="mstream", bufs=2))
            mps = est.enter_context(tc.tile_pool(name="mps", bufs=4,
                                                 space="PSUM"))
            mpss = est.enter_context(tc.tile_pool(name="mpss", bufs=2,
                                                  space="PSUM"))
            wb_s = mpool.tile([128, 17 * 640], f16, name="wb_s")
            nc.sync.dma_start(wb_s[:], wb_d[:])
            esc = mpool.tile([16, 3], f32, name="esc")
            nc.scalar.activation(esc[:], sabf[:], AF.Exp)
            z0m = mpool.tile([GPC, FT + 1], f16, name="z0m")
            nc.sync.dma_start(z0m[:], z0_d[:])

            def trans_blocks(zp, kb):
                """zp [16, kb*128] f16 -> zT [128, kb*16] f16 via PE."""
                zT = msb.tile([128, kb * 16], f16, tag="zT", name="zT")
                for k in range(kb):
                    pt = mpss.tile([128, 16], f16, tag="mtr", name="pt")
                    nc.tensor.transpose(pt[:], zp[:, k * 128:(k + 1) * 128],
                                        ident_s[:16, :16])
                    nc.scalar.activation(zT[:, k * 16:(k + 1) * 16], pt[:],
                                         AF.Copy)
                return zT

            def mm_thin(zT, kb, w_s, ncols):
                """out [16, ncols] f32 = zT.T @ w; w_s view [128, kb, ncols]."""
                w_v = w_s[:].rearrange("p (k n) -> p k n", k=kb)
                out = msb.tile([16, ncols], f32, tag="mlpo", name="out")
                zT_v = zT[:].rearrange("p (k n) -> p k n", k=kb)
                for o in range(0, ncols, 512):
                    n = min(512, ncols - o)
                    pm = mps.tile([128, 512], f32, tag="mbig", name="pm")
                    for k in range(kb):
                        nc.tensor.matmul(pm[:16, :n], lhsT=zT_v[:, k, :],
                                         rhs=w_v[:, k, o:o + n],
                                         start=(k == 0), stop=(k == kb - 1))
                    nc.scalar.activation(out[:, o:o + n], pm[:16, :n], AF.Copy)
                return out

            def llin_post(zz, kout, esc_idx):
                t1 = msb.tile([16, 1], f32, tag="t1", name="t1", bufs=8)
                nc.scalar.activation(t1[:], zz[:, 0:1], AF.Sigmoid)
                nc.vector.tensor_scalar(
                    out=t1[:], in0=t1[:],
                    scalar1=esc[:, esc_idx:esc_idx + 1],
                    scalar2=1.1, op0=ALU.mult, op1=ALU.add)
                sp = zz[:, 1:kout]
                sq = msb.tile([16, kout - 1], f32, tag="msq", name="sq")
                ac = msb.tile([16, 1], f32, tag="t1", name="ac", bufs=8)
                nc.scalar.activation(sq[:], sp, AF.Square, accum_out=ac[:])
                nc.vector.tensor_scalar_max(ac[:], ac[:], 1e-8)
                r_ = msb.tile([16, 1], f32, tag="t1", name="r_", bufs=8)
                nc.vector.reciprocal(r_[:], ac[:])
                t2 = msb.tile([16, 1], f32, tag="t1", name="t2", bufs=8)
                nc.vector.tensor_tensor(out=t2[:], in0=t1[:], in1=t1[:],
                                        op=ALU.mult)
                nc.vector.tensor_scalar_add(t2[:], t2[:], -1.0)
                nc.vector.tensor_tensor(out=r_[:], in0=r_[:], in1=t2[:],
                                        op=ALU.mult)
                nsqrt(r_[:], r_[:], msb, "m_")
                return t1, r_

            # llin-a: z0 [16, 513] -> zA [16, 2560(junk pad)]
            z0p = msb.tile([16, 640], f16, tag="z0p", name="z0p")
            nc.scalar.activation(z0p[:, 0:513], z0m[:], AF.Copy)
            nc.vector.memset(z0p[:, 513:514], 1.0)
            nc.vector.memset(z0p[:, 514:640], 0.0)
            zT = trans_blocks(z0p, 5)
            zA = mm_thin(zT, 5, wa_s, 2560)
            t1, r1 = llin_post(zA, 2049, 0)
            z1p = msb.tile([16, 17 * 128], f16, tag="z1p", name="z1p")
            nc.scalar.activation(z1p[:, 1:2049], zA[:, 1:2049], AF.Gelu,
                                 scale=r1[:])
            sqz = msb.tile([16, 2048], f32, tag="msq", name="sqz")
            az = msb.tile([16, 1], f32, tag="t1", name="az", bufs=8)
            nc.scalar.activation(sqz[:], z1p[:, 1:2049], AF.Square,
                                 accum_out=az[:])
            az1 = msb.tile([16, 1], f32, tag="t1", name="az1", bufs=8)
            nc.scalar.activation(az1[:], az[:], AF.Identity, bias=1.0)
            nsqrt(z1p[:, 0:1], az1[:], msb, "z_")
            nc.vector.memset(z1p[:, 2049:2050], 1.0)
            nc.vector.memset(z1p[:, 2050:2176], 0.0)
            # llin-b: [16, 2049] -> [16, 513]
            zTb = trans_blocks(z1p, 17)
            zB = mm_thin(zTb, 17, wb_s, 640)
            t3, r3 = llin_post(zB, 513, 1)
            z2p = msb.tile([16, 640], f16, tag="z0p", name="z2p")
            nc.scalar.activation(z2p[:, 0:1], t3[:], AF.Copy)
            nc.scalar.activation(z2p[:, 1:513], zB[:, 1:513], AF.Copy,
                                 scale=r3[:])
            nc.vector.memset(z2p[:, 513:514], 1.0)
            nc.vector.memset(z2p[:, 514:640], 0.0)
            # llin-f: [16, 513] -> [16, 513]
            zTf = trans_blocks(z2p, 5)
            zF = mm_thin(zTf, 5, wf_s, 640)
            t4, r4 = llin_post(zF, 513, 2)
            zfin = msb.tile([16, FT + 1], f32, tag="mlpo", name="zfin")
            nc.scalar.activation(zfin[:, 0:1], t4[:], AF.Copy)
            nc.scalar.activation(zfin[:, 1:513], zF[:, 1:513], AF.Copy,
                                 scale=r4[:])
            nc.sync.dma_start(zout[:], zfin[:])

    nc.compile()
    return nc


# ============================ host entry ============================

EX_DTYPE = "bfloat16"   # safe exp range


def _make_inmaps(inputs):
    x = np.asarray(inputs["x"], np.float32)
    edge_index = np.asarray(inputs["edge_index"])
    srcs, sdt, sjt, idx, LP, NJ = _prep_edges(edge_index)

    f16 = np.float16
    f8np = ml_dtypes.float8_e4m3

    def aug5(W, b):
        return _interleave_k(
            _aug_w(np.asarray(W, np.float32), np.asarray(b, np.float32), 640), 5)

    w1l_h = aug5(inputs["Wl1"], inputs["bl1"]).astype(f16).reshape(128, 5 * FT)
    w1r_h = aug5(inputs["Wr1"], inputs["br1"]).astype(f16).reshape(128, 5 * FT)
    w2l_h = _interleave_k(np.asarray(inputs["Wl2"], np.float32), 4
                          ).astype(f16).reshape(128, 4 * FT)
    w2r_h = _interleave_k(np.asarray(inputs["Wr2"], np.float32), 4
                          ).astype(f16).reshape(128, 4 * FT)
    b2rows = np.stack([np.asarray(inputs["bl2"]),
                       np.asarray(inputs["br2"])]).astype(f16)

    def amat(att):
        att = np.asarray(att, np.float32)
        A = np.zeros((FT, HEADS), np.float32)
        for hh in range(HEADS):
            A[hh * C:(hh + 1) * C, hh] = att[hh]
        return _interleave_k(A, 4).astype(f16).reshape(128, 4 * HEADS)

    a1_h, a2_h = amat(inputs["att1"]), amat(inputs["att2"])
    b1full = np.tile(np.asarray(inputs["bias1"], f16)[None, :], (128, 1))
    b2full = np.tile(np.asarray(inputs["bias2"], f16)[None, :], (128, 1))
    ecols = np.zeros((128, NT * GPC), f16)
    for t in range(NT):
        ecols[:, t * GPC + t] = 1.0
    ident = np.eye(128, dtype=f16)
    wa_h = _interleave_k(_aug_w(np.asarray(inputs["Wa"], np.float32),
                                np.asarray(inputs["ba"], np.float32), 640, 2560),
                         5).astype(f16).reshape(128, 5 * 2560)
    wb_h = _interleave_k(_aug_w(np.asarray(inputs["Wb"], np.float32),
                                np.asarray(inputs["bb"], np.float32),
                                17 * 128, 640), 17
                         ).astype(f16).reshape(128, 17 * 640)
    wf_h = _interleave_k(_aug_w(np.asarray(inputs["Wf"], np.float32),
                                np.asarray(inputs["bf"], np.float32), 640, 640),
                         5).astype(f16).reshape(128, 5 * 640)
    sabf = np.tile(np.array([[float(inputs["sa"]), float(inputs["sb"]),
                              float(inputs["sf"])]], np.float32), (16, 1))

    # full augmented-transposed x, shared by all cores
    xT = np.zeros((640, N), np.float32)
    xT[:FT] = x[:, 1:].T
    xT[FT] = 1.0
    xfT_h = _interleave_k(xT, 5).astype(f16).reshape(128, 5 * N)

    in_maps = []
    for k in range(NCORES):
        sl = slice(k * SHARD, (k + 1) * SHARD)
        xTo = np.zeros((640, SHARD), np.float32)
        xTo[:FT] = x[sl, 1:].T
        xTo[FT] = 1.0
        xoT_h = _interleave_k(xTo, 5).astype(f16).reshape(128, 5 * SHARD)
        tsl = slice(k * NT, (k + 1) * NT)
        in_maps.append({
            "xfT": xfT_h, "xoT": xoT_h,
            "w1l": w1l_h, "w1r": w1r_h, "w2l": w2l_h, "w2r": w2r_h,
            "b2rows": b2rows, "a1": a1_h, "a2": a2_h, "b1full": b1full,
            "b2full": b2full,
            "sdt": sdt[tsl].astype(f8np),
            "sj": np.ascontiguousarray(
                sjt[tsl].reshape(NT, NJ, 128, 128).transpose(0, 2, 1, 3)
            ).reshape(NT, 128, NJ * 128).astype(f8np),
            "idx": idx[tsl],
            "ecols": ecols, "ident": ident,
            "wa": wa_h, "wb": wb_h, "wf": wf_h, "sabf": sabf,
        })

    return in_maps, LP, NJ


_last_exec_ns = None


def kernel(**inputs):
    global _last_exec_ns
    in_maps, LP, NJ = _make_inmaps(inputs)
    key = (LP, EX_DTYPE)
    if key not in _cache:
        _cache[key] = _build(LP, NJ, EX_DTYPE)
    nc = _cache[key]
    from concourse.bass_utils import run_bass_kernel_spmd
    res = run_bass_kernel_spmd(nc, in_maps, list(range(NCORES)))
    _last_exec_ns = res.exec_time_ns
    kernel._last_res = res
    z = np.concatenate([np.asarray(r["zout"]) for r in res.results], 0)
    gm = np.concatenate([np.asarray(r["gmout"]) for r in res.results], 0)
    return z.astype(np.float32), gm.astype(np.float32)


# revision 43
# speedup vs baseline: 1.1086x; 1.1086x over previous
"""Trainium2 Bass kernel for nn_LorentzGNN (2x GATv2 + Lorentz head), 8-core SPMD.

Sharding: nodes (and their in-edges) are partitioned contiguously across 8 cores
(2048 nodes each). The layer-1 source transform (xl1 table) is REPLICATED on
every core (cheaper than an AllGather of the table); layer-2 takes one
AllGather of xl2. Per-edge work uses gpsimd dma_gather (edge-major + feat-major
transposed) and host-built 0/1 segment matrices (fp8) driven through the PE as
matmuls. Leaky-relu is a single Prelu activation (co-resident with Exp in one
act table -> no table reloads in the edge phase). Gelu and the Lorentz sqrt are
deferred to batched passes for the same reason. Graph phase (centroid + Lorentz
MLP over the 16 graphs each core owns) is local; host concatenates.
"""
import numpy as np
import ml_dtypes

# ---------- problem constants (hardcoded per contract) ----------
N, E, B = 16384, 131072, 128
FT, HEADS, C = 512, 4, 128
NCORES = 8
SHARD = N // NCORES            # 2048
P = 128
NT = SHARD // P                # 16 dst tiles per core
NTALL = N // P                 # 128 tiles overall
GPC = B // NCORES              # 16 graphs per core
LEAK = 0.2

_cache = {}


# ============================ host-side prep ============================

def _prep_edges(edge_index):
    src = np.concatenate([edge_index[0], np.arange(N)]).astype(np.int64)
    dst = np.concatenate([edge_index[1], np.arange(N)]).astype(np.int64)
    order = np.argsort(dst, kind="stable")
    src, dst = src[order], dst[order]
    counts = np.bincount(dst // P, minlength=NTALL)
    LP = int(np.ceil(max(counts.max(), 1) / 128) * 128)
    NJ = LP // P
    srcs = np.zeros((NTALL, LP), np.int16)
    dloc = np.full((NTALL, LP), -1, np.int32)
    starts = np.concatenate([[0], np.cumsum(counts)])
    for t in range(NTALL):
        c = counts[t]
        srcs[t, :c] = src[starts[t]:starts[t] + c]
        dloc[t, :c] = dst[starts[t]:starts[t] + c] - t * P
    # segment matrices
    sdt = np.zeros((NTALL, P, LP), np.float16)           # [dst, j]
    jj = np.arange(LP)
    for t in range(NTALL):
        v = dloc[t] >= 0
        sdt[t, dloc[t, v], jj[v]] = 1.0
    sjt = np.ascontiguousarray(sdt.transpose(0, 2, 1))   # [j, dst] edge-major
    # idx buffers wrapped in 16 partitions, replicated to 128
    idx = srcs.reshape(NTALL, LP // 16, 16).transpose(0, 2, 1)  # [t, 16, LP/16]
    idx = np.tile(idx, (1, 8, 1)).astype(np.int16)              # [t, 128, LP/16]
    return srcs, sdt, sjt, idx, LP, NJ


def _interleave_k(w, kchunks):
    """[K*128, N] -> [128, K, N] host layout for SBUF."""
    K, Nn = w.shape
    assert K == kchunks * 128
    return np.ascontiguousarray(w.reshape(kchunks, 128, Nn).transpose(1, 0, 2))


def _aug_w(W, b, kpad, npad=None):
    """stack rows [W; b; 0-pad] to kpad rows, optionally pad cols to npad."""
    K, Nn = W.shape
    out = np.zeros((kpad, Nn if npad is None else npad), np.float32)
    out[:K, :Nn] = W
    out[K, :Nn] = b
    return out


# ============================ kernel build ============================

def _build(LP, NJ, ex_dtype_name):
    import contextlib
    import concourse.bass as bass  # noqa
    import concourse.bacc as bacc
    import concourse.tile as tile
    from concourse import mybir
    from concourse.library_config import mlp as gpsimd_mlp

    f32, f16 = mybir.dt.float32, mybir.dt.float16
    bf16, i16 = mybir.dt.bfloat16, mybir.dt.int16
    f8 = mybir.dt.float8e4
    EXD = {"float16": f16, "bfloat16": bf16}[ex_dtype_name]
    AF = mybir.ActivationFunctionType
    ALU = mybir.AluOpType
    SC = [(o, min(512, LP - o)) for o in range(0, LP, 512)]

    nc = bacc.Bacc("TRN2", target_bir_lowering=False, debug=False,
                   num_devices=NCORES)
    groups = [list(range(NCORES))]

    # ---- DRAM I/O (per-core, same program) ----
    xfT = nc.dram_tensor("xfT", [128, 4 * N], f16, kind="ExternalInput")
    xoT = nc.dram_tensor("xoT", [128, 4 * SHARD], f16, kind="ExternalInput")
    w1l = nc.dram_tensor("w1l", [128, 4 * FT], f16, kind="ExternalInput")
    w1r = nc.dram_tensor("w1r", [128, 4 * FT], f16, kind="ExternalInput")
    b1r_d = nc.dram_tensor("b1rows", [2, FT], f16, kind="ExternalInput")
    w2l = nc.dram_tensor("w2l", [128, 4 * FT], f16, kind="ExternalInput")
    w2r = nc.dram_tensor("w2r", [128, 4 * FT], f16, kind="ExternalInput")
    b2r_d = nc.dram_tensor("b2rows", [2, FT], f16, kind="ExternalInput")
    a1_d = nc.dram_tensor("a1", [128, 4 * HEADS], f16, kind="ExternalInput")
    a2_d = nc.dram_tensor("a2", [128, 4 * HEADS], f16, kind="ExternalInput")
    b1f_d = nc.dram_tensor("b1full", [128, FT], f16, kind="ExternalInput")
    b2f_d = nc.dram_tensor("b2full", [128, FT], f16, kind="ExternalInput")
    sdt_d = nc.dram_tensor("sdt", [NT, 128, LP], f8, kind="ExternalInput")
    sj_d = nc.dram_tensor("sj", [NT, 128, NJ * 128], f8, kind="ExternalInput")
    idx_d = nc.dram_tensor("idx", [NT, 128, LP // 16], i16, kind="ExternalInput")
    ecols_d = nc.dram_tensor("ecols", [128, NT * GPC], f16, kind="ExternalInput")
    ident_d = nc.dram_tensor("ident", [128, 128], f16, kind="ExternalInput")
    wa_d = nc.dram_tensor("wa", [128, 5 * 2560], f16, kind="ExternalInput")
    wb_d = nc.dram_tensor("wb", [128, 17 * 640], f16, kind="ExternalInput")
    wf_d = nc.dram_tensor("wf", [128, 5 * 640], f16, kind="ExternalInput")
    sabf_d = nc.dram_tensor("sabf", [16, 3], f32, kind="ExternalInput")

    xl1_tb = nc.dram_tensor("xl1_tb", [N, FT], f16)
    xl2_sh = nc.dram_tensor("xl2_sh", [SHARD, FT], f16)
    xl2_tb = nc.dram_tensor("xl2_tb", [N, FT], f16, addr_space="Shared")
    z0_d = nc.dram_tensor("z0buf", [GPC, FT + 1], f16)
    za_d = nc.dram_tensor("zabuf", [1, GPC], f32)
    zout = nc.dram_tensor("zout", [GPC, FT + 1], f32, kind="ExternalOutput")
    gmout = nc.dram_tensor("gmout", [GPC, FT + 1], f32, kind="ExternalOutput")

    with tile.TileContext(nc, num_cores=NCORES) as tc:
        est = contextlib.ExitStack()
        with est:
            nc.gpsimd.load_library(gpsimd_mlp)
            nregs = {n: nc.gpsimd.to_reg(n) for n in sorted({n for _, n in SC})}
            cpool = est.enter_context(tc.tile_pool(name="consts", bufs=1))
            resp = est.enter_context(tc.tile_pool(name="res", bufs=1))
            mearly = est.enter_context(tc.tile_pool(name="mearly", bufs=1))
            big = contextlib.ExitStack()
            xsp = big.enter_context(tc.tile_pool(name="xstream", bufs=2))
            sbp = big.enter_context(tc.tile_pool(name="stream", bufs=2))
            smp = big.enter_context(tc.tile_pool(name="small", bufs=2))
            psb = big.enter_context(tc.tile_pool(name="psb", bufs=2, space="PSUM"))
            pgp = big.enter_context(tc.tile_pool(name="pgp", bufs=1, space="PSUM"))
            pss = big.enter_context(tc.tile_pool(name="pss", bufs=2, space="PSUM"))

            # ---- consts ----
            def cload(name, shape, dt_, src):
                t = cpool.tile(shape, dt_, name=name)
                nc.sync.dma_start(t[:], src)
                return t

            w1l_s = cload("w1l_s", [128, 4 * FT], f16, w1l[:])
            w1r_s = cload("w1r_s", [128, 4 * FT], f16, w1r[:])
            b1la_s = cload("b1la_s", [1, FT], f16, b1r_d[0:1, :])
            b1ra_s = cload("b1ra_s", [1, FT], f16, b1r_d[1:2, :])
            w2l_s = cload("w2l_s", [128, 4 * FT], f16, w2l[:])
            w2r_s = cload("w2r_s", [128, 4 * FT], f16, w2r[:])
            b2la_s = cload("b2la_s", [1, FT], f16, b2r_d[0:1, :])
            b2ra_s = cload("b2ra_s", [1, FT], f16, b2r_d[1:2, :])
            a1_s = cload("a1_s", [128, 4 * HEADS], f16, a1_d[:])
            a2_s = cload("a2_s", [128, 4 * HEADS], f16, a2_d[:])
            b1f_s = cload("b1f_s", [128, FT], f16, b1f_d[:])
            b2f_s = cload("b2f_s", [128, FT], f16, b2f_d[:])
            ident_s = cload("ident_s", [128, 128], f16, ident_d[:])
            ecols_s = cload("ecols_s", [128, NT * GPC], f16, ecols_d[:])
            ones1 = cpool.tile([1, FT], f16, name="ones1")
            nc.vector.memset(ones1[:], 1.0)
            onesc = cpool.tile([128, 1], f32, name="onesc")
            nc.vector.memset(onesc[:], 1.0)

            def nsqrt(out_ap, x_ap, pool, pfx):
                """out = sqrt(x), Newton-refined (ACT sqrt LUT is ~4e-3)."""
                y0 = pool.tile(list(x_ap.shape), f32, tag="nsq", name=pfx + "y0",
                               bufs=6)
                nc.scalar.activation(y0[:], x_ap, AF.Sqrt)
                r0 = pool.tile(list(x_ap.shape), f32, tag="nsq", name=pfx + "r0",
                               bufs=6)
                nc.vector.reciprocal(r0[:], y0[:])
                nc.vector.tensor_tensor(out=r0[:], in0=x_ap, in1=r0[:],
                                        op=ALU.mult)
                nc.vector.tensor_tensor(out=y0[:], in0=y0[:], in1=r0[:],
                                        op=ALU.add)
                nc.vector.tensor_scalar_mul(out_ap, y0[:], 0.5)

            # resident tiles
            xr_s = resp.tile([128, NT * FT], f16, name="xr_s")
            h1T_s = resp.tile([128, NT * FT], f16, name="h1T_s")
            h2_s = resp.tile([128, NT * (FT + 1)], f16, name="h2_s")
            acb_s = resp.tile([128, NT], f32, name="acb_s")
            h2_v = h2_s[:].rearrange("p (t c) -> p t c", t=NT)

            # ---------------- transform1: replicated xl1 table ----------------
            def transform1():
                w1l_v = w1l_s[:].rearrange("p (k n) -> p k n", k=4)
                w1r_v = w1r_s[:].rearrange("p (k n) -> p k n", k=4)
                xfT_v = xfT[:].rearrange("p (k n) -> p k n", k=4)
                CH = 4  # tiles per streamed chunk
                for c0 in range(0, NTALL, CH):
                    xc = xsp.tile([128, 4 * CH * 128], f16, tag="xc", name="xc")
                    xc_v = xc[:].rearrange("p (k n) -> p k n", k=4)
                    nc.sync.dma_start(xc_v, xfT_v[:, :, c0 * 128:(c0 + CH) * 128])
                    for ti in range(CH):
                        t = c0 + ti
                        pz2 = psb.tile([128, 1024], f32, tag="pz2", name="pl1")
                        pl = pz2[:, 0:FT]
                        for kc in range(4):
                            lh = xc_v[:, kc, ti * 128:(ti + 1) * 128]
                            nc.tensor.matmul(pl, lhsT=lh, rhs=w1l_v[:, kc, :],
                                             start=(kc == 0), stop=False)
                        nc.tensor.matmul(pl, lhsT=ones1[:, 0:128],
                                         rhs=b1la_s[:], start=False, stop=True)
                        xlt = smp.tile([128, FT], f16, tag="xlt", name="xlt",
                                       bufs=8)
                        if t % 2 == 0:
                            nc.scalar.activation(xlt[:], pl, AF.Copy)
                        else:
                            nc.vector.tensor_copy(xlt[:], pl)
                        nc.sync.dma_start(xl1_tb[t * 128:(t + 1) * 128, :], xlt[:])
                # own-shard xr1 from the per-core xoT input
                xoT_v = xoT[:].rearrange("p (k n) -> p k n", k=4)
                for c0 in range(0, NT, CH):
                    xc = xsp.tile([128, 4 * CH * 128], f16, tag="xc", name="xco")
                    xc_v = xc[:].rearrange("p (k n) -> p k n", k=4)
                    nc.sync.dma_start(xc_v, xoT_v[:, :, c0 * 128:(c0 + CH) * 128])
                    for ti in range(CH):
                        t = c0 + ti
                        pz2 = psb.tile([128, 1024], f32, tag="pz2", name="pr1")
                        pr = pz2[:, 0:FT]
                        for kc in range(4):
                            lh = xc_v[:, kc, ti * 128:(ti + 1) * 128]
                            nc.tensor.matmul(pr, lhsT=lh, rhs=w1r_v[:, kc, :],
                                             start=(kc == 0), stop=False)
                        nc.tensor.matmul(pr, lhsT=ones1[:, 0:128],
                                         rhs=b1ra_s[:], start=False, stop=True)
                        if t % 2 == 0:
                            nc.scalar.activation(xr_s[:, t * FT:(t + 1) * FT],
                                                 pr, AF.Copy)
                        else:
                            nc.vector.tensor_copy(xr_s[:, t * FT:(t + 1) * FT],
                                                  pr)

            # ---------------- transform2 (local shard) ----------------
            # h1T_s is produced (pre-gelu) inline in edge_layer(1); a batched
            # gelu pass then runs, interleaved with transform2_xl.
            def transform2_xl(tiles):
                """xl2 = h1T.T @ W2l + bl2 -> xl2_sh."""
                w2l_v = w2l_s[:].rearrange("p (k n) -> p k n", k=4)
                for t in tiles:
                    h1T_v = h1T_s[:, t * FT:(t + 1) * FT].rearrange(
                        "p (k n) -> p k n", k=4)
                    pz2 = psb.tile([128, 1024], f32, tag="pz2", name="pl2")
                    pl = pz2[:, 0:FT]
                    for kc in range(4):
                        nc.tensor.matmul(pl, lhsT=h1T_v[:, kc, :],
                                         rhs=w2l_v[:, kc, :],
                                         start=(kc == 0), stop=False)
                    nc.tensor.matmul(pl, lhsT=ones1[:, 0:128], rhs=b2la_s[:],
                                     start=False, stop=True)
                    xlt = smp.tile([128, FT], f16, tag="xlt", name="xlt2",
                                   bufs=8)
                    if t % 2 == 0:
                        nc.scalar.activation(xlt[:], pl, AF.Copy)
                    else:
                        nc.vector.tensor_copy(xlt[:], pl)
                    nc.sync.dma_start(xl2_sh[t * 128:(t + 1) * 128, :], xlt[:])

            def transform2_xr():
                """xr2 = h1T.T @ W2r + br2 -> xr_s (overlaps the AllGather)."""
                w2r_v = w2r_s[:].rearrange("p (k n) -> p k n", k=4)
                for t in range(NT):
                    h1T_v = h1T_s[:, t * FT:(t + 1) * FT].rearrange(
                        "p (k n) -> p k n", k=4)
                    pz2 = psb.tile([128, 1024], f32, tag="pz2", name="pr2")
                    pr = pz2[:, 0:FT]
                    for kc in range(4):
                        nc.tensor.matmul(pr, lhsT=h1T_v[:, kc, :],
                                         rhs=w2r_v[:, kc, :],
                                         start=(kc == 0), stop=False)
                    nc.tensor.matmul(pr, lhsT=ones1[:, 0:128], rhs=b2ra_s[:],
                                     start=False, stop=True)
                    if t % 2 == 0:
                        nc.scalar.activation(xr_s[:, t * FT:(t + 1) * FT], pr,
                                             AF.Copy)
                    else:
                        nc.vector.tensor_copy(xr_s[:, t * FT:(t + 1) * FT], pr)

            # ---------------- GATv2 edge layer ----------------
            def edge_layer(layer, table, a_s, bf_s):
                for t in range(NT):
                    idxt = smp.tile([128, LP // 16], i16, tag="idxt", name="idxt")
                    nc.sync.dma_start(idxt[:], idx_d[t, :, :])
                    sdtt = sbp.tile([128, LP], f8, tag="sdtt", name="sdtt")
                    nc.sync.dma_start(sdtt[:], sdt_d[t, :, :])
                    sjt = sbp.tile([128, NJ * 128], f8, tag="sjt", name="sjt")
                    nc.sync.dma_start(sjt[:], sj_d[t, :, :])
                    sj_v = sjt[:].rearrange("p (j d) -> p j d", j=NJ)

                    # gathers: xlgT feat-major (SC-block layout [4c, n]),
                    # xlg edge-major [jb, 512]
                    xlgT = sbp.tile([128, 4 * LP], f16, tag="xlgT", name="xlgT")
                    xlg = sbp.tile([128, NJ * FT], f16, tag="xlg", name="xlg")
                    xlg_w = xlg[:].rearrange("p (j n) -> p j n", j=NJ)
                    for (o, n) in SC:
                        nc.gpsimd.dma_gather(
                            xlgT[:, 4 * o:4 * (o + n)].rearrange(
                                "p (c j) -> p c j", c=4), table[:],
                            idxt[:, o // 16:(o + n) // 16], n, nregs[n], FT,
                            transpose=True)
                        nc.gpsimd.dma_gather(
                            xlg_w[:, o // 128:(o + n) // 128, :], table[:],
                            idxt[:, o // 16:(o + n) // 16], n, nregs[n], FT)
                    xlg_v = xlg[:].rearrange("p (j n) -> p j n", j=NJ)

                    xr_t = xr_s[:].rearrange("p (t n) -> p t n", t=NT)[:, t, :]
                    # z = xl[src] + xr[dst]; fc-chunks paired into one 2-bank
                    # psum so each Prelu covers 2n columns.
                    lr = sbp.tile([128, 4 * LP], f16, tag="lr", name="lr")
                    for (o, n) in SC:
                        for fcp in (0, 2):
                            pz2 = psb.tile([128, 1024], f32, tag="pz2",
                                           name="pz2")
                            for fi in range(2):
                                fc = fcp + fi
                                pz = pz2[:, fi * 512:fi * 512 + n]
                                nc.tensor.matmul(
                                    pz, lhsT=xr_t[:, fc * 128:(fc + 1) * 128],
                                    rhs=sdtt[:, o:o + n], start=True, stop=False)
                                nc.tensor.matmul(
                                    pz, lhsT=ident_s[:],
                                    rhs=xlgT[:, 4 * o + fc * n:
                                             4 * o + (fc + 1) * n],
                                    start=False, stop=True)
                            if n == 512:
                                nc.scalar.activation(
                                    lr[:, 4 * o + fcp * n:4 * o + (fcp + 2) * n],
                                    pz2[:], AF.Prelu, alpha=LEAK)
                            else:
                                for fi in range(2):
                                    fc = fcp + fi
                                    nc.scalar.activation(
                                        lr[:, 4 * o + fc * n:4 * o + (fc + 1) * n],
                                        pz2[:, fi * 512:fi * 512 + n],
                                        AF.Prelu, alpha=LEAK)
                    # attention logits: att is head-block-diagonal, so chunk fc
                    # contributes only to head h=fc -> single-column matmuls.
                    a_v = a_s[:].rearrange("p (c h) -> p c h", c=4)
                    ps_sm = pss.tile([128, 512], f32, tag="ps", name="ps_sm")
                    pe_all = ps_sm[:, 0:NJ * HEADS]
                    pden = ps_sm[:, 128:128 + HEADS]
                    for jb in range(NJ):
                        o = 512 * (jb // 4)
                        n = min(512, LP - o)
                        joff = (jb % 4) * 128
                        for h in range(HEADS):
                            lblk = lr[:, 4 * o + h * n + joff:
                                      4 * o + h * n + joff + 128]
                            nc.tensor.matmul(
                                pe_all[:, jb * HEADS + h:jb * HEADS + h + 1],
                                lhsT=lblk, rhs=a_v[:, h, h:h + 1],
                                start=True, stop=True)
                    exf = smp.tile([128, NJ * HEADS], f32, tag="exf", name="exf")
                    nc.scalar.activation(exf[:], pe_all, AF.Exp)
                    exb = smp.tile([128, NJ * HEADS], EXD, tag="exb", name="exb")
                    nc.vector.tensor_copy(exb[:], exf[:])

                    pagg = pgp.tile([128, FT], f32, tag="pagg", name="pagg",
                                    bufs=1)
                    for jb in range(NJ):
                        wt = sbp.tile([128, FT], EXD, tag="wt", name="wt")
                        for h in range(HEADS):
                            nc.vector.tensor_scalar_mul(
                                wt[:, h * C:(h + 1) * C],
                                xlg_v[:, jb, h * C:(h + 1) * C],
                                exf[:, jb * HEADS + h:jb * HEADS + h + 1])
                        nc.tensor.matmul(pagg[:], lhsT=sj_v[:, jb], rhs=wt[:],
                                         start=(jb == 0), stop=(jb == NJ - 1))
                        nc.tensor.matmul(pden, lhsT=sj_v[:, jb],
                                         rhs=exb[:, jb * HEADS:(jb + 1) * HEADS],
                                         start=(jb == 0), stop=(jb == NJ - 1))
                    rden = smp.tile([128, HEADS], f32, tag="rden", name="rden")
                    nc.vector.reciprocal(rden[:], pden)
                    o1 = smp.tile([128, FT], f16, tag="o1", name="o1")
                    for h in range(HEADS):
                        nc.vector.tensor_scalar_mul(
                            o1[:, h * C:(h + 1) * C], pagg[:, h * C:(h + 1) * C],
                            rden[:, h:h + 1])
                    if layer == 1:
                        # h1pre = o1 + b1; transpose into h1T_s inline (gelu
                        # deferred to a batched pass over h1T_s)
                        h1p = smp.tile([128, FT], f16, tag="h1p", name="h1p")
                        nc.vector.tensor_tensor(out=h1p[:], in0=o1[:],
                                                in1=bf_s[:], op=ALU.add)
                        for fc in range(4):
                            pt = psb.tile([128, 128], f16, tag="pt", name="pt",
                                          bufs=1)
                            nc.tensor.transpose(
                                pt[:], h1p[:, fc * 128:(fc + 1) * 128],
                                ident_s[:])
                            nc.vector.tensor_copy(
                                h1T_s[:, t * FT + fc * 128:
                                      t * FT + (fc + 1) * 128], pt[:])
                    else:
                        nc.vector.tensor_tensor(
                            out=h2_v[:, t, 1:FT + 1], in0=o1[:], in1=bf_s[:],
                            op=ALU.add)
                        sq = smp.tile([128, FT], f16, tag="sq", name="sq")
                        acc = smp.tile([128, 1], f32, tag="acc", name="acc")
                        nc.scalar.activation(sq[:], h2_v[:, t, 1:FT + 1],
                                             AF.Square, accum_out=acc[:])
                        nc.vector.tensor_scalar_add(acb_s[:, t:t + 1], acc[:],
                                                    1.0)
                        # z0 spatial part: row 0 of this tile == graph t's
                        # first node
                        nc.sync.dma_start(z0_d[t:t + 1, 1:FT + 1],
                                          h2_v[0:1, t, 1:FT + 1])

            # ---------------- program ----------------
            transform1()
            edge_layer(1, xl1_tb, a1_s, b1f_s)
            # gelu (on the transposed copy, one table load) interleaved with
            # the xl2 transform so the AllGather can issue as early as possible
            GB = 4 * FT
            for g in range(4):
                nc.scalar.activation(h1T_s[:, g * GB:(g + 1) * GB],
                                     h1T_s[:, g * GB:(g + 1) * GB], AF.Gelu)
                transform2_xl(range(4 * g, 4 * g + 4))
            nc.gpsimd.collective_compute(
                "AllGather", mybir.AluOpType.bypass, replica_groups=groups,
                ins=[xl2_sh[:]], outs=[xl2_tb[:]])
            # prefetch MLP weights + overlap xr2 under the AllGather
            wa_s = mearly.tile([128, 5 * 2560], f16, name="wa_s")
            nc.sync.dma_start(wa_s[:], wa_d[:])
            wf_s = mearly.tile([128, 5 * 640], f16, name="wf_s")
            nc.sync.dma_start(wf_s[:], wf_d[:])
            sabf = mearly.tile([16, 3], f32, name="sabf")
            nc.sync.dma_start(sabf[:], sabf_d[:])
            transform2_xr()
            edge_layer(2, xl2_tb, a2_s, b2f_s)
            # deferred Lorentz time coordinate: t = sqrt(1 + |sp|^2)
            tvals = smp.tile([128, NT], f32, tag="tv", name="tvals")
            nsqrt(tvals[:], acb_s[:], smp, "t_")
            # z0 time col: stage row 0 of tvals through DRAM ([1,16] store,
            # then a standard partition-scattering [16,1] load)
            nc.sync.dma_start(za_d[:], tvals[0:1, :])
            zt16 = smp.tile([GPC, 1], f32, tag="in1", name="zt16")
            nc.sync.dma_start(
                zt16[:], za_d[:].rearrange("o (g x) -> (o g) x", x=1))
            zt16h = smp.tile([GPC, 1], f16, tag="in1h", name="zt16h")
            nc.vector.tensor_copy(zt16h[:], zt16[:])
            nc.sync.dma_start(z0_d[:, 0:1], zt16h[:])

            # ---------------- graph phase: centroid ----------------
            pgm = pgp.tile([128, FT], f32, tag="pagg", name="pgm", bufs=1)
            for t in range(NT):
                ec = ecols_s[:, t * GPC:(t + 1) * GPC]
                nc.tensor.matmul(pgm[:GPC, :], lhsT=ec, rhs=h2_v[:, t, 1:FT + 1],
                                 start=(t == 0), stop=(t == NT - 1))
            # time-coordinate sums: graph g == tile g, so they are the column
            # sums of tvals
            pgm1 = pss.tile([128, 512], f32, tag="ps", name="pgm1")
            nc.tensor.matmul(pgm1[0:NT, 0:1], lhsT=tvals[:], rhs=onesc[:],
                             start=True, stop=True)
            sums = smp.tile([GPC, FT + 1], f32, tag="sums", name="sums")
            nc.scalar.activation(sums[:, 1:FT + 1], pgm[:GPC, :], AF.Copy)
            nc.scalar.activation(sums[:, 0:1], pgm1[:GPC, 0:1], AF.Copy)
            sqg = smp.tile([GPC, FT], f32, tag="sqg", name="sqg")
            sa_ = smp.tile([GPC, 1], f32, tag="acc", name="sa_")
            nc.scalar.activation(sqg[:], sums[:, 1:FT + 1], AF.Square,
                                 accum_out=sa_[:])
            innr = smp.tile([GPC, 1], f32, tag="in1", name="innr")
            nc.vector.tensor_tensor(out=innr[:], in0=sums[:, 0:1],
                                    in1=sums[:, 0:1], op=ALU.mult)
            nc.vector.tensor_tensor(out=innr[:], in0=innr[:], in1=sa_[:],
                                    op=ALU.subtract)
            nc.vector.tensor_scalar_max(innr[:], innr[:], 1e-8 * (N // B) ** 2)
            rt = smp.tile([GPC, 1], f32, tag="in1", name="rt")
            nsqrt(rt[:], innr[:], smp, "g_")
            nc.vector.reciprocal(rt[:], rt[:])
            gmt = smp.tile([GPC, FT + 1], f32, tag="sums", name="gmt")
            nc.scalar.activation(gmt[:], sums[:], AF.Copy, scale=rt[:])
            nc.sync.dma_start(gmout[:], gmt[:])

            # free the big stream/psum pools before the MLP pools allocate
            big.close()

            # ---------------- Lorentz MLP on z0 [16, 513] ----------------
            mpool = est.enter_context(tc.tile_pool(name="mlp", bufs=1))
            msb = est.enter_context(tc.tile_pool(name="mstream", bufs=2))
            mps = est.enter_context(tc.tile_pool(name="mps", bufs=4,
                                                 space="PSUM"))
            mpss = est.enter_context(tc.tile_pool(name="mpss", bufs=2,
                                                  space="PSUM"))
            wb_s = mpool.tile([128, 17 * 640], f16, name="wb_s")
            nc.sync.dma_start(wb_s[:], wb_d[:])
            esc = mpool.tile([16, 3], f32, name="esc")
            nc.scalar.activation(esc[:], sabf[:], AF.Exp)
            z0m = mpool.tile([GPC, FT + 1], f16, name="z0m")
            nc.sync.dma_start(z0m[:], z0_d[:])

            def trans_blocks(zp, kb):
                """zp [16, kb*128] f16 -> zT [128, kb*16] f16 via PE."""
                zT = msb.tile([128, kb * 16], f16, tag="zT", name="zT")
                for k in range(kb):
                    pt = mpss.tile([128, 16], f16, tag="mtr", name="pt")
                    nc.tensor.transpose(pt[:], zp[:, k * 128:(k + 1) * 128],
                                        ident_s[:16, :16])
                    nc.scalar.activation(zT[:, k * 16:(k + 1) * 16], pt[:],
                                         AF.Copy)
                return zT

            def mm_thin(zT, kb, w_s, ncols):
                """out [16, ncols] f32 = zT.T @ w; w_s view [128, kb, ncols]."""
                w_v = w_s[:].rearrange("p (k n) -> p k n", k=kb)
                out = msb.tile([16, ncols], f32, tag="mlpo", name="out")
                zT_v = zT[:].rearrange("p (k n) -> p k n", k=kb)
                for o in range(0, ncols, 512):
                    n = min(512, ncols - o)
                    pm = mps.tile([128, 512], f32, tag="mbig", name="pm")
                    for k in range(kb):
                        nc.tensor.matmul(pm[:16, :n], lhsT=zT_v[:, k, :],
                                         rhs=w_v[:, k, o:o + n],
                                         start=(k == 0), stop=(k == kb - 1))
                    nc.scalar.activation(out[:, o:o + n], pm[:16, :n], AF.Copy)
                return out

            def llin_post(zz, kout, esc_idx):
                t1 = msb.tile([16, 1], f32, tag="t1", name="t1", bufs=8)
                nc.scalar.activation(t1[:], zz[:, 0:1], AF.Sigmoid)
                nc.vector.tensor_scalar(
                    out=t1[:], in0=t1[:],
                    scalar1=esc[:, esc_idx:esc_idx + 1],
                    scalar2=1.1, op0=ALU.mult, op1=ALU.add)
                sp = zz[:, 1:kout]
                sq = msb.tile([16, kout - 1], f32, tag="msq", name="sq")
                ac = msb.tile([16, 1], f32, tag="t1", name="ac", bufs=8)
                nc.scalar.activation(sq[:], sp, AF.Square, accum_out=ac[:])
                nc.vector.tensor_scalar_max(ac[:], ac[:], 1e-8)
                r_ = msb.tile([16, 1], f32, tag="t1", name="r_", bufs=8)
                nc.vector.reciprocal(r_[:], ac[:])
                t2 = msb.tile([16, 1], f32, tag="t1", name="t2", bufs=8)
                nc.vector.tensor_tensor(out=t2[:], in0=t1[:], in1=t1[:],
                                        op=ALU.mult)
                nc.vector.tensor_scalar_add(t2[:], t2[:], -1.0)
                nc.vector.tensor_tensor(out=r_[:], in0=r_[:], in1=t2[:],
                                        op=ALU.mult)
                nsqrt(r_[:], r_[:], msb, "m_")
                return t1, r_

            # llin-a: z0 [16, 513] -> zA [16, 2560(junk pad)]
            z0p = msb.tile([16, 640], f16, tag="z0p", name="z0p")
            nc.scalar.activation(z0p[:, 0:513], z0m[:], AF.Copy)
            nc.vector.memset(z0p[:, 513:514], 1.0)
            nc.vector.memset(z0p[:, 514:640], 0.0)
            zT = trans_blocks(z0p, 5)
            zA = mm_thin(zT, 5, wa_s, 2560)
            t1, r1 = llin_post(zA, 2049, 0)
            z1p = msb.tile([16, 17 * 128], f16, tag="z1p", name="z1p")
            nc.scalar.activation(z1p[:, 1:2049], zA[:, 1:2049], AF.Gelu,
                                 scale=r1[:])
            sqz = msb.tile([16, 2048], f32, tag="msq", name="sqz")
            az = msb.tile([16, 1], f32, tag="t1", name="az", bufs=8)
            nc.scalar.activation(sqz[:], z1p[:, 1:2049], AF.Square,
                                 accum_out=az[:])
            az1 = msb.tile([16, 1], f32, tag="t1", name="az1", bufs=8)
            nc.scalar.activation(az1[:], az[:], AF.Identity, bias=1.0)
            nsqrt(z1p[:, 0:1], az1[:], msb, "z_")
            nc.vector.memset(z1p[:, 2049:2050], 1.0)
            nc.vector.memset(z1p[:, 2050:2176], 0.0)
            # llin-b: [16, 2049] -> [16, 513]
            zTb = trans_blocks(z1p, 17)
            zB = mm_thin(zTb, 17, wb_s, 640)
            t3, r3 = llin_post(zB, 513, 1)
            z2p = msb.tile([16, 640], f16, tag="z0p", name="z2p")
            nc.scalar.activation(z2p[:, 0:1], t3[:], AF.Copy)
            nc.scalar.activation(z2p[:, 1:513], zB[:, 1:513], AF.Copy,
                                 scale=r3[:])
            nc.vector.memset(z2p[:, 513:514], 1.0)
            nc.vector.memset(z2p[:, 514:640], 0.0)
            # llin-f: [16, 513] -> [16, 513]
            zTf = trans_blocks(z2p, 5)
            zF = mm_thin(zTf, 5, wf_s, 640)
            t4, r4 = llin_post(zF, 513, 2)
            zfin = msb.tile([16, FT + 1], f32, tag="mlpo", name="zfin")
            nc.scalar.activation(zfin[:, 0:1], t4[:], AF.Copy)
            nc.scalar.activation(zfin[:, 1:513], zF[:, 1:513], AF.Copy,
                                 scale=r4[:])
            nc.sync.dma_start(zout[:], zfin[:])

    nc.compile()
    return nc


# ============================ host entry ============================

EX_DTYPE = "bfloat16"   # safe exp range


def _make_inmaps(inputs):
    x = np.asarray(inputs["x"], np.float32)
    edge_index = np.asarray(inputs["edge_index"])
    srcs, sdt, sjt, idx, LP, NJ = _prep_edges(edge_index)

    f16 = np.float16
    f8np = ml_dtypes.float8_e4m3

    w1l_h = _interleave_k(np.asarray(inputs["Wl1"], np.float32), 4
                          ).astype(f16).reshape(128, 4 * FT)
    w1r_h = _interleave_k(np.asarray(inputs["Wr1"], np.float32), 4
                          ).astype(f16).reshape(128, 4 * FT)
    b1rows = np.stack([np.asarray(inputs["bl1"]),
                       np.asarray(inputs["br1"])]).astype(f16)
    w2l_h = _interleave_k(np.asarray(inputs["Wl2"], np.float32), 4
                          ).astype(f16).reshape(128, 4 * FT)
    w2r_h = _interleave_k(np.asarray(inputs["Wr2"], np.float32), 4
                          ).astype(f16).reshape(128, 4 * FT)
    b2rows = np.stack([np.asarray(inputs["bl2"]),
                       np.asarray(inputs["br2"])]).astype(f16)

    def amat(att):
        att = np.asarray(att, np.float32)
        A = np.zeros((FT, HEADS), np.float32)
        for hh in range(HEADS):
            A[hh * C:(hh + 1) * C, hh] = att[hh]
        return _interleave_k(A, 4).astype(f16).reshape(128, 4 * HEADS)

    a1_h, a2_h = amat(inputs["att1"]), amat(inputs["att2"])
    b1full = np.tile(np.asarray(inputs["bias1"], f16)[None, :], (128, 1))
    b2full = np.tile(np.asarray(inputs["bias2"], f16)[None, :], (128, 1))
    ecols = np.zeros((128, NT * GPC), f16)
    for t in range(NT):
        ecols[:, t * GPC + t] = 1.0
    ident = np.eye(128, dtype=f16)
    wa_h = _interleave_k(_aug_w(np.asarray(inputs["Wa"], np.float32),
                                np.asarray(inputs["ba"], np.float32), 640, 2560),
                         5).astype(f16).reshape(128, 5 * 2560)
    wb_h = _interleave_k(_aug_w(np.asarray(inputs["Wb"], np.float32),
                                np.asarray(inputs["bb"], np.float32),
                                17 * 128, 640), 17
                         ).astype(f16).reshape(128, 17 * 640)
    wf_h = _interleave_k(_aug_w(np.asarray(inputs["Wf"], np.float32),
                                np.asarray(inputs["bf"], np.float32), 640, 640),
                         5).astype(f16).reshape(128, 5 * 640)
    sabf = np.tile(np.array([[float(inputs["sa"]), float(inputs["sb"]),
                              float(inputs["sf"])]], np.float32), (16, 1))

    # full transposed x, shared by all cores
    xfT_h = _interleave_k(np.ascontiguousarray(x[:, 1:].T), 4
                          ).astype(f16).reshape(128, 4 * N)

    in_maps = []
    for k in range(NCORES):
        sl = slice(k * SHARD, (k + 1) * SHARD)
        xoT_h = _interleave_k(np.ascontiguousarray(x[sl, 1:].T), 4
                              ).astype(f16).reshape(128, 4 * SHARD)
        tsl = slice(k * NT, (k + 1) * NT)
        in_maps.append({
            "xfT": xfT_h, "xoT": xoT_h,
            "w1l": w1l_h, "w1r": w1r_h, "w2l": w2l_h, "w2r": w2r_h,
            "b1rows": b1rows, "b2rows": b2rows, "a1": a1_h, "a2": a2_h, "b1full": b1full,
            "b2full": b2full,
            "sdt": sdt[tsl].astype(f8np),
            "sj": np.ascontiguousarray(
                sjt[tsl].reshape(NT, NJ, 128, 128).transpose(0, 2, 1, 3)
            ).reshape(NT, 128, NJ * 128).astype(f8np),
            "idx": idx[tsl],
            "ecols": ecols, "ident": ident,
            "wa": wa_h, "wb": wb_h, "wf": wf_h, "sabf": sabf,
        })

    return in_maps, LP, NJ


_last_exec_ns = None


def kernel(**inputs):
    global _last_exec_ns
    in_maps, LP, NJ = _make_inmaps(inputs)
    key = (LP, EX_DTYPE)
    if key not in _cache:
        _cache[key] = _build(LP, NJ, EX_DTYPE)
    nc = _cache[key]
    from concourse.bass_utils import run_bass_kernel_spmd
    res = run_bass_kernel_spmd(nc, in_maps, list(range(NCORES)))
    _last_exec_ns = res.exec_time_ns
    kernel._last_res = res
    z = np.concatenate([np.asarray(r["zout"]) for r in res.results], 0)
    gm = np.concatenate([np.asarray(r["gmout"]) for r in res.results], 0)
    return z.astype(np.float32), gm.astype(np.float32)
